# revision 54
# baseline (speedup 1.0000x reference)
"""Trainium2 Bass kernel for attribute visual attention.

Computes, for each batch b:
    q      = v @ W_alpha                  # [i, f]
    scores = q @ vf[b]                    # [i, r]
    atten  = softmax(scores, axis=r)
    out[b] = atten @ vf[b].T              # [i, f]

Sharding: data-parallel over batch b across 8 NeuronCores (8 batches per
core); v / W_alpha replicated.

Design (v2 — transposed-scores pipeline):
- scores are computed TRANSPOSED: scoresT[r, i] = vf[b].T-contract via
  lhsT = vf[b] in its natural [f, r] layout, rhs = qT[f, i]. The softmax
  then needs no on-chip transposes at all: exp runs on scoresT directly
  (partition dim = r), and the attend matmul consumes esT[r, i] as the
  moving operand with lhsT = vfT[b] slices.
- No per-row max subtraction. scores*1.0 - 30.0 feeds Exp; results are
  stored in bf16 (range to 3e38 absorbs exp(~86) tails, and the +/-30
  shift cancels in the normalization). The softmax denominator comes from
  a ones-matmul that broadcasts the per-column sum to all 128 partitions
  in PSUM for free; normalization is a single DVE multiply on the small
  esT tiles (atten stored fp16 once normalized, values <= 1).
- PE work is software-pipelined as scoresT(k) -> attend(k-1) -> sums(k)
  so the tensor engine never waits on the ACT/DVE softmax chain.
- All inputs are DMA'd up front (everything fits in SBUF); inputs ride
  the HWDGE (sync) queue, outputs the SWDGE (gpsimd) queue.
- Attend outputs drain from PSUM via dual-bank copies (two 312-col tiles
  per instruction, alternating scalar/vector engines).
"""

import numpy as np
from contextlib import ExitStack

import concourse.bass as bass
import concourse.tile as tile
import concourse.bass_utils as bass_utils
from concourse import bacc, mybir

# Problem shapes (hardcoded per contest contract).
B, F, R, I, V = 64, 2048, 196, 312, 300
NCORES = 8
BL = B // NCORES          # 8 batches per core
FT = F // 128             # 16 f-tiles
KV_TILES = ((0, 128), (128, 128), (256, 44))    # v=300
KR_TILES = ((0, 128), (128, 68))                # r=196
EXP_SHIFT = -30.0

F16 = mybir.dt.float16
BF16 = mybir.dt.bfloat16
F32 = mybir.dt.float32

_CACHE = {}


WQ = I + F     # waq packed width: [vT | W_alpha]
WH = WQ // 2   # half-column DMA chunk


def _build_body(nc, tc, ctx, waq, vfp, vft, out, reps):
    constp = ctx.enter_context(tc.tile_pool(name="const", bufs=1))
    vfpp = ctx.enter_context(tc.tile_pool(name="vfp", bufs=1))
    vftp = ctx.enter_context(tc.tile_pool(name="vft", bufs=1))

    ones = constp.tile([128, 128], BF16, tag="ones")
    ebias = constp.tile([128, 1], F32, tag="ebias")
    nc.vector.memset(ones[:], 1.0)
    nc.vector.memset(ebias[:], EXP_SHIFT)

    # ---- all input DMAs up front (everything is SBUF-resident).
    # Weights ride SP's HWDGE queue; the bulk vf loads ride the Pool/SWDGE
    # queue whose descriptor generator then has nothing else to do, so the
    # per-batch supply stays ahead of per-batch demand.  Outputs use SP. ----
    wa_t = {}
    with tc.high_priority():
        for h in range(2):
            for k, (v0, vs) in enumerate(KV_TILES):
                if h == 0:
                    w = constp.tile([vs, WQ], F16, tag=f"wa{k}", name=f"wa{k}")
                    wa_t[k] = w
                # three parallel HWDGE issue streams so the weight chunks are
                # not paced by a single SEQ's per-copy issue latency
                eng = (nc.sync, nc.scalar, nc.sync)[k]
                eng.dma_start(wa_t[k][:, h * WH:(h + 1) * WH],
                              waq[v0:v0 + vs, h * WH:(h + 1) * WH])
    vfp_t, vft_t = [], {}
    for b in range(BL):
        t = vfpp.tile([128, FT, R], F16, tag=f"vfp{b}")
        # early batches in t-chunks so the weight DMAs aren't stuck behind a
        # monolithic transfer and the fused batch-0 scores can start early
        nch = 4 if b == 0 else (2 if b == 1 else 1)
        step = FT // nch
        for c in range(nch):
            nc.gpsimd.dma_start(t[:, c * step:(c + 1) * step, :],
                                vfp[b, :, c * step:(c + 1) * step, :])
        vfp_t.append(t)
        for kr, (r0, rs) in enumerate(KR_TILES):
            v = vftp.tile([rs, F], F16, tag=f"vft{b}_{kr}")
            for hh in range(2):
                nc.gpsimd.dma_start(v[:, hh * 1024:(hh + 1) * 1024],
                                    vft[b, r0:r0 + rs, hh * 1024:(hh + 1) * 1024])
            vft_t[(b, kr)] = v

    # ---- PE warm-up while the first weight chunks land ----
    with tc.tile_pool(name="wupsum", bufs=1, space=bass.MemorySpace.PSUM) as wup:
        wu = wup.tile([128, 128], F32, tag="wu")
        for w in range(32):
            nc.tensor.matmul(wu[:], ones[:], ones[:],
                             start=(w == 0), stop=(w == 31))

    # ---- Phase 1: per-batch attention, PE-pipelined ----
    esp = ctx.enter_context(tc.tile_pool(name="es", bufs=6))
    attp = ctx.enter_context(tc.tile_pool(name="att", bufs=6))
    rcpp = ctx.enter_context(tc.tile_pool(name="rcp", bufs=3))
    outp = ctx.enter_context(tc.tile_pool(name="out", bufs=3))
    spsum = ctx.enter_context(
        tc.tile_pool(name="spsum", bufs=1, space=bass.MemorySpace.PSUM))
    smpsum = ctx.enter_context(
        tc.tile_pool(name="smpsum", bufs=1, space=bass.MemorySpace.PSUM))

    # ---- Phase 0 fused with batch 0's scores: the qT tiles are produced
    # two steps ahead of their use by scoresT(0), so batch 0's scores finish
    # ~right after the last q tile instead of a full scores-pass later. ----
    qt_t = []
    sp0 = [spsum.tile([rs, I], F32, tag=f"sp{kr}", name=f"sp{kr}")
           for kr, (r0, rs) in enumerate(KR_TILES)]

    def scores_step(b, sp, kf):
        for kr, (r0, rs) in enumerate(KR_TILES):
            nc.tensor.matmul(sp[kr][:], vfp_t[b][:, kf, r0:r0 + rs],
                             qt_t[kf][:], start=(kf == 0),
                             stop=(kf == FT - 1))

    def exp_es(sp):
        es_t = []
        for kr, (r0, rs) in enumerate(KR_TILES):
            es = esp.tile([rs, I], BF16, tag=f"es{kr}", name=f"es{kr}")
            with tc.high_priority():
                nc.scalar.activation(es[:], sp[kr][:],
                                     mybir.ActivationFunctionType.Exp,
                                     bias=ebias[0:rs, :], scale=1.0)
            es_t.append(es)
        return es_t

    with tc.tile_pool(name="qpsum", bufs=2, space=bass.MemorySpace.PSUM) as qpsum:
        for mf in range(FT):
            qp = qpsum.tile([128, I], F32, tag="qp")
            for k, (v0, vs) in enumerate(KV_TILES):
                nc.tensor.matmul(
                    qp[:], wa_t[k][:, I + mf * 128:I + (mf + 1) * 128],
                    wa_t[k][:, 0:I], start=(k == 0), stop=(k == 2))
            q = constp.tile([128, I], F16, tag=f"qt{mf}")
            nc.vector.tensor_copy(q[:], qp[:])
            qt_t.append(q)
            if mf >= 2:
                scores_step(0, sp0, mf - 2)
        scores_step(0, sp0, FT - 2)
        scores_step(0, sp0, FT - 1)
    es0 = exp_es(sp0)
    opsum = ctx.enter_context(
        tc.tile_pool(name="opsum", bufs=5, space=bass.MemorySpace.PSUM))

    def scores_exp(b, mid=None):
        es_t = []
        for kr, (r0, rs) in enumerate(KR_TILES):
            sp = spsum.tile([rs, I], F32, tag=f"sp{kr}", name=f"sp{kr}")
            for kf in range(FT):
                nc.tensor.matmul(sp[:], vfp_t[b][:, kf, r0:r0 + rs],
                                 qt_t[kf][:], start=(kf == 0),
                                 stop=(kf == FT - 1))
            es = esp.tile([rs, I], BF16, tag=f"es{kr}", name=f"es{kr}")
            with tc.high_priority():
                nc.scalar.activation(es[:], sp[:],
                                     mybir.ActivationFunctionType.Exp,
                                     bias=ebias[0:rs, :], scale=1.0)
            es_t.append(es)
            if kr == 0 and mid is not None:
                mid()
        return es_t

    def sums_rcp_norm(b, es_t):
        sm = smpsum.tile([128, I], F32, tag="sm", name="sm")
        for kr, (r0, rs) in enumerate(KR_TILES):
            nc.tensor.matmul(sm[:], ones[0:rs, :], es_t[kr][:],
                             start=(kr == 0), stop=(kr == 1))
        rcpb = rcpp.tile([128, I], F32, tag="rcpb", name="rcpb")
        att_t = []
        with tc.high_priority():
            nc.vector.reciprocal(rcpb[:], sm[:])
            for kr, (r0, rs) in enumerate(KR_TILES):
                at = attp.tile([rs, I], F16, tag=f"at{kr}", name=f"at{kr}")
                nc.vector.tensor_tensor(at[:], es_t[kr][:], rcpb[0:rs, :],
                                        mybir.AluOpType.mult)
                att_t.append(at)
        return att_t

    def attend_part(b, att_t, otf, mfs, chunks, flip=False):
        for mf in mfs:
            op_ = opsum.tile([128, 512], F32, tag="op", name="op")
            for kr, (r0, rs) in enumerate(KR_TILES):
                nc.tensor.matmul(
                    op_[:, 0:I],
                    vft_t[(b, kr)][:, mf * 128:(mf + 1) * 128],
                    att_t[kr][:], start=(kr == 0), stop=(kr == 1))
            dst = otf[:, mf, :]
            src = op_[:, 0:I]
            if (mf % 2 == 0) != flip:
                nc.scalar.copy(dst, src)
            else:
                nc.vector.tensor_copy(dst, src)
            if mf in chunks:
                c0, cn, eng = chunks[mf]
                eng.dma_start(out[b, :, c0:c0 + cn, :],
                              otf[:, c0:c0 + cn, :])

    CH4 = {3: (0, 4), 7: (4, 4), 11: (8, 4), 15: (12, 4)}

    def attend(b, att_t, last=False):
        if last:
            chunks = {3: (0, 4, nc.sync), 7: (4, 4, nc.sync),
                      11: (8, 4, nc.sync), 13: (12, 2, nc.scalar),
                      15: (14, 2, nc.sync)}
        else:
            chunks = {mf: (c0, cn, nc.sync) for mf, (c0, cn) in CH4.items()}
        otf = outp.tile([128, FT, I], F16, tag="otf", name="otf")
        attend_part(b, att_t, otf, range(FT), chunks)

    for rep in range(reps):
        prev = None   # (b, att_t)
        for b in range(BL):
            if rep == 0 and b == 0:
                continue   # batch 0's scores were fused; sums deferred to b=1
            if rep == 0 and b == 1:
                # emit sums(0) between scoresT(1)'s two kr groups so the PE
                # has work while exp(0) finishes (nothing else fills it yet)
                hold = {}
                es_t = scores_exp(1, mid=lambda: hold.update(
                    a=sums_rcp_norm(0, es0)))
                prev = (0, hold["a"])
            else:
                es_t = scores_exp(b)
            if prev is None:
                att_t = sums_rcp_norm(b, es_t)
            elif b == BL - 1:
                # split the previous attend around this batch's sums so the
                # PE has work covering the rcp/normalize latency (there is no
                # scoresT(b+1) left to hide it behind)
                pb, patt = prev
                otf = outp.tile([128, FT, I], F16, tag="otf", name="otf")
                ch = {mf: (c0, cn, nc.sync) for mf, (c0, cn) in CH4.items()}
                attend_part(pb, patt, otf, range(0, 8), ch)
                att_t = sums_rcp_norm(b, es_t)
                attend_part(pb, patt, otf, range(8, FT), ch)
            else:
                attend(*prev)
                att_t = sums_rcp_norm(b, es_t)
            prev = (b, att_t)
        attend(prev[0], prev[1], last=(rep == reps - 1))


def _get_program(reps=1):
    key = ("nc", reps)
    if key in _CACHE:
        return _CACHE[key]
    nc = bacc.Bacc("TRN2", target_bir_lowering=False, debug=False,
                   num_devices=NCORES)
    waq_d = nc.dram_tensor("waq", [V, WQ], F16, kind="ExternalInput")
    vfp_d = nc.dram_tensor("vfp", [BL, 128, FT, R], F16, kind="ExternalInput")
    vft_d = nc.dram_tensor("vft", [BL, R, F], F16, kind="ExternalInput")
    out_d = nc.dram_tensor("out", [BL, 128, FT, I], F16,
                           kind="ExternalOutput")

    with tile.TileContext(nc) as tc, ExitStack() as ctx:
        _build_body(nc, tc, ctx, waq_d.ap(), vfp_d.ap(),
                    vft_d.ap(), out_d.ap(), reps)
    nc.compile()
    _CACHE[key] = nc
    return nc


def _prep_inputs(visual_features, v, W_alpha):
    vf = np.asarray(visual_features, dtype=np.float32)
    v = np.asarray(v, dtype=np.float32)
    W = np.asarray(W_alpha, dtype=np.float32)

    # packed [vT | W_alpha]: [V, I + F]
    waq16 = np.ascontiguousarray(
        np.concatenate([v.T, W], axis=1)).astype(np.float16)
    # [b, f, r] -> [b, p=128, t=16, r]  (f = t*128 + p)
    vfp16 = np.ascontiguousarray(
        vf.reshape(B, FT, 128, R).transpose(0, 2, 1, 3)).astype(np.float16)
    vft16 = np.ascontiguousarray(vf.transpose(0, 2, 1)).astype(np.float16)

    in_maps = []
    for c in range(NCORES):
        in_maps.append({
            "waq": waq16,
            "vfp": np.ascontiguousarray(vfp16[c * BL:(c + 1) * BL]),
            "vft": np.ascontiguousarray(vft16[c * BL:(c + 1) * BL]),
        })
    return in_maps


def kernel(visual_features, v, W_alpha):
    nc = _get_program()
    in_maps = _prep_inputs(visual_features, v, W_alpha)
    res = None
    for attempt in range(3):
        try:
            res = bass_utils.run_bass_kernel_spmd(
                nc, in_maps, core_ids=list(range(NCORES)))
            break
        except Exception:
            # transient NRT_EXEC_UNIT_UNRECOVERABLE wedges have been seen on
            # this fabric; a re-dispatch typically succeeds
            if attempt == 2:
                raise
    outs = [res.results[c]["out"] for c in range(NCORES)]
    buf = np.concatenate(outs, axis=0)          # [B, p=128, t=16, I]
    full = buf.transpose(0, 3, 2, 1).reshape(B, I, F)   # f = t*128 + p
    return np.ascontiguousarray(full).astype(np.float32)
